# revision 9
# baseline (speedup 1.0000x reference)
"""ChebyKAN linear layer on 8 Trainium2 NeuronCores.

Computation: out[b,o] = sum_{i,d} T_d(tanh(x[b,i])) * coef[i,o,d]
  == sum_d T_d(tanh(x)) @ C_d   (9 accumulated 8192x1024x1024 matmuls)

Strategy:
  - Data-parallel over batch: core c handles rows [c*1024, (c+1)*1024).
  - Host pre-transposes each core's x slice to (in_features, batch) layout so
    the contraction dim (i) lands on SBUF partitions, and repacks the
    coefficients to (d, i, o) bf16.
  - On-chip: ACT computes tanh in fp32, DVE runs the Chebyshev recursion
    T_d = 2 t T_{d-1} - T_{d-2} in fp32 (scalar_tensor_tensor fuses the
    2*t*T_{d-1} product into one op), ACT casts each T_d to bf16, and PE
    accumulates the 8 degree-matmuls (d=1..8) in fp32 PSUM.
  - The d=0 term (T_0 == 1) is folded on the host into a single extra
    128-contraction "bias" matmul: W_bias[k,o] = sum_j C_0[j*128+k, o],
    multiplied by an all-ones stationary tile.
  - Per core the 1024-row batch is processed in two 512-column halves; each
    half keeps its full output (4 b-chunks x 2 o-halves) resident in all
    8 PSUM banks while 65 k-blocks accumulate into it.

Numerics (validated in numpy): rel l2 error vs fp32 reference ~2e-3.
"""

import numpy as np
import ml_dtypes

BATCH = 8192
IN_F = 1024
OUT_F = 1024
DEG = 8  # degree; DEG+1 coefficients per (i,o)
N_CORES = 8
B_CORE = BATCH // N_CORES  # 1024
P = 128
HALF = 512  # batch columns processed per PSUM-resident output block
NI = IN_F // P  # 8 contraction tiles
NBC = HALF // P  # 4 b-chunks per half
NOH = OUT_F // 512  # 2 output halves of 512
N_HALF = B_CORE // HALF  # 2

_CACHED_NC = {}


def _build_bass(loop_r=None):
    """Build the Bass program. loop_r wraps the whole compute in a hardware
    For loop of loop_r iterations (benchmark-only; slope over loop_r gives
    per-iteration HW time since the axon RPC overhead is per-call)."""
    import contextlib

    import concourse.mybir as mybir
    import concourse.tile as tile
    from concourse import bacc

    f32 = mybir.dt.float32
    bf16 = mybir.dt.bfloat16
    mult = mybir.AluOpType.mult
    sub = mybir.AluOpType.subtract
    Tanh = mybir.ActivationFunctionType.Tanh

    nc = bacc.Bacc(name="chebykan")
    xt = nc.dram_tensor("xt", (IN_F, B_CORE), f32, kind="ExternalInput")
    w = nc.dram_tensor("w", (DEG, IN_F, OUT_F), bf16, kind="ExternalInput")
    wb = nc.dram_tensor("wb", (P, OUT_F), bf16, kind="ExternalInput")
    out = nc.dram_tensor("out", (B_CORE, OUT_F), f32, kind="ExternalOutput")

    with (
        tile.TileContext(nc) as tc,
        tc.tile_pool(name="wpool", bufs=10) as wpool,
        tc.tile_pool(name="xpool", bufs=8) as xpool,
        tc.tile_pool(name="tanh", bufs=3) as tanpool,
        tc.tile_pool(name="rec", bufs=6) as rpool,
        tc.tile_pool(name="ch", bufs=16) as chpool,
        tc.tile_pool(name="const", bufs=1) as cpool,
        tc.tile_pool(name="outp", bufs=8) as opool,
        tc.tile_pool(name="psum", bufs=1, space="PSUM") as pspool,
    ):
        ones = cpool.tile([P, P], bf16)
        nc.vector.memset(ones[:], 1.0)
        wbias = cpool.tile([P, OUT_F], bf16)
        nc.sync.dma_start(wbias[:], wb[:, :])

        loop_cm = (
            tc.For_i(
                0,
                loop_r,
                1,
                hint_engines=(mybir.EngineType.PE, mybir.EngineType.SP),
            )
            if loop_r is not None
            else contextlib.nullcontext()
        )
        with loop_cm:
            _emit_body(nc, tc, xt, w, out, ones, wbias,
                       wpool, xpool, tanpool, rpool, chpool, opool, pspool,
                       f32, bf16, mult, sub, Tanh)
    nc.finalize()
    return nc


def _emit_body(nc, tc, xt, w, out, ones, wbias,
               wpool, xpool, tanpool, rpool, chpool, opool, pspool,
               f32, bf16, mult, sub, Tanh):
    for h in range(N_HALF):
            ps = [
                [
                    pspool.tile(
                        [P, 512], f32, tag=f"ps_{bc}_{oh}", name=f"ps_{bc}_{oh}"
                    )
                    for oh in range(NOH)
                ]
                for bc in range(NBC)
            ]
            # Bias k-block: out += ones.T @ W_bias (covers the d=0 term).
            # start=True clears the PSUM banks.
            for bc in range(NBC):
                for oh in range(NOH):
                    nc.tensor.matmul(
                        ps[bc][oh],
                        ones,
                        wbias[:, oh * 512 : (oh + 1) * 512],
                        start=True,
                        stop=False,
                    )
            for i in range(NI):
                xti = xpool.tile([P, HALF], f32, tag="x")
                nc.sync.dma_start(
                    xti[:], xt[i * P : (i + 1) * P, h * HALF : (h + 1) * HALF]
                )
                t = tanpool.tile([P, HALF], f32, tag="t")
                nc.scalar.activation(t[:], xti[:], Tanh)

                tm2 = None  # T_{d-2} (fp32); None encodes T_0 == 1
                tm1 = t  # T_{d-1} (fp32)
                for d in range(1, DEG + 1):
                    last = d == DEG
                    chd = chpool.tile([P, HALF], bf16, tag="ch")
                    if d == 1:
                        nc.scalar.copy(chd[:], t[:])
                        cur = t
                    else:
                        # pr = (T_{d-1} * 2) * t  (one fused DVE op)
                        pr = rpool.tile([P, HALF], f32, tag="rec")
                        nc.vector.scalar_tensor_tensor(
                            pr[:], tm1[:], 2.0, t[:], mult, mult
                        )
                        if d == 2:
                            # T_2 = pr - 1
                            cur = rpool.tile([P, HALF], f32, tag="rec")
                            nc.vector.tensor_scalar_sub(cur[:], pr[:], 1.0)
                            nc.scalar.copy(chd[:], cur[:])
                        elif not last:
                            cur = rpool.tile([P, HALF], f32, tag="rec")
                            nc.vector.tensor_tensor(cur[:], pr[:], tm2[:], sub)
                            nc.scalar.copy(chd[:], cur[:])
                        else:
                            # final degree: write the bf16 tile directly
                            cur = None
                            nc.vector.tensor_tensor(chd[:], pr[:], tm2[:], sub)
                    tm2, tm1 = tm1, cur

                    wt = wpool.tile([P, OUT_F], bf16, tag="w")
                    nc.sync.dma_start(wt[:], w[d - 1, i * P : (i + 1) * P, :])
                    stop = i == NI - 1 and d == DEG
                    for bc in range(NBC):
                        lhsT = chd[:, bc * P : (bc + 1) * P]
                        for oh in range(NOH):
                            nc.tensor.matmul(
                                ps[bc][oh],
                                lhsT,
                                wt[:, oh * 512 : (oh + 1) * 512],
                                start=False,
                                stop=stop,
                            )
            # Drain this half's PSUM to SBUF and then HBM. Copies alternate
            # between DVE and ACT to halve the bank-free latency.
            for bc in range(NBC):
                for oh in range(NOH):
                    ot = opool.tile([P, 512], f32, tag="ot")
                    if (bc * NOH + oh) % 2 == 0:
                        nc.vector.tensor_copy(ot[:], ps[bc][oh])
                    else:
                        nc.scalar.copy(ot[:], ps[bc][oh])
                    r0 = h * HALF + bc * P
                    nc.sync.dma_start(
                        out[r0 : r0 + P, oh * 512 : (oh + 1) * 512], ot[:]
                    )


def _get_nc(loop_r=None):
    if loop_r not in _CACHED_NC:
        _CACHED_NC[loop_r] = _build_bass(loop_r)
    return _CACHED_NC[loop_r]


def _prep_inputs(x, coefficients):
    bf16 = ml_dtypes.bfloat16
    x = np.asarray(x, dtype=np.float32)
    coef = np.asarray(coefficients, dtype=np.float32)
    # (d, i, o) bf16 for d = 1..DEG
    w_all = np.ascontiguousarray(coef.transpose(2, 0, 1)[1 : DEG + 1]).astype(bf16)
    # d=0 term folded over i into a single 128-row contraction block
    wb_arr = np.ascontiguousarray(
        coef[:, :, 0].reshape(NI, P, OUT_F).sum(axis=0)
    ).astype(bf16)
    in_maps = []
    for c in range(N_CORES):
        xc = x[c * B_CORE : (c + 1) * B_CORE, :]
        in_maps.append(
            {
                "xt": np.ascontiguousarray(xc.T),
                "w": w_all,
                "wb": wb_arr,
            }
        )
    return in_maps


def run(x, coefficients, trace=False, tmpdir=None):
    """Run on hardware; returns (out, BassKernelResults)."""
    from concourse.bass_utils import run_bass_kernel_spmd

    nc = _get_nc()
    in_maps = _prep_inputs(x, coefficients)
    res = run_bass_kernel_spmd(
        nc,
        in_maps,
        core_ids=list(range(N_CORES)),
        trace=trace,
        tmpdir=tmpdir,
    )
    out = np.concatenate([r["out"] for r in res.results], axis=0)
    return np.ascontiguousarray(out, dtype=np.float32), res


def kernel(x, coefficients):
    out, _ = run(x, coefficients, trace=False)
    return out


# revision 23
# speedup vs baseline: 1.0225x; 1.0225x over previous
"""ChebyKAN linear layer on 8 Trainium2 NeuronCores.

Computation: out[b,o] = sum_{i,d} T_d(tanh(x[b,i])) * coef[i,o,d]
  == sum_d T_d(tanh(x)) @ C_d   (9 accumulated 8192x1024x1024 matmuls)

Strategy:
  - Data-parallel over batch: core c handles rows [c*1024, (c+1)*1024).
  - Host pre-transposes each core's x slice to (in_features, batch) layout so
    the contraction dim (i) lands on SBUF partitions, and repacks the
    coefficients to (d, i, o) bf16.
  - On-chip: ACT computes tanh in fp32, DVE runs the Chebyshev recursion
    T_d = 2 t T_{d-1} - T_{d-2} in fp32 (scalar_tensor_tensor fuses the
    2*t*T_{d-1} product into one op), ACT casts each T_d to bf16, and PE
    accumulates the 8 degree-matmuls (d=1..8) in fp32 PSUM.
  - The d=0 term (T_0 == 1) is folded on the host into a single extra
    128-contraction "bias" matmul: W_bias[k,o] = sum_j C_0[j*128+k, o],
    multiplied by an all-ones stationary tile.
  - Per core the 1024-row batch is processed in two 512-column halves; each
    half keeps its full output (4 b-chunks x 2 o-halves) resident in all
    8 PSUM banks while 65 k-blocks accumulate into it.

Numerics (validated in numpy): rel l2 error vs fp32 reference ~2e-3.
"""

import numpy as np
import ml_dtypes

BATCH = 8192
IN_F = 1024
OUT_F = 1024
DEG = 8  # degree; DEG+1 coefficients per (i,o)
N_CORES = 8
B_CORE = BATCH // N_CORES  # 1024
P = 128
HALF = 512  # batch columns processed per PSUM-resident output block
NI = IN_F // P  # 8 contraction tiles
NBC = HALF // P  # 4 b-chunks per half
NOH = OUT_F // 512  # 2 output halves of 512
N_HALF = B_CORE // HALF  # 2

_CACHED_NC = {}


def _build_bass(loop_r=None, variant=""):
    """Build the Bass program. loop_r wraps the whole compute in a hardware
    For loop of loop_r iterations (benchmark-only; slope over loop_r gives
    per-iteration HW time since the axon RPC overhead is per-call)."""
    import contextlib

    import concourse.mybir as mybir
    import concourse.tile as tile
    from concourse import bacc

    f32 = mybir.dt.float32
    bf16 = mybir.dt.bfloat16
    mult = mybir.AluOpType.mult
    sub = mybir.AluOpType.subtract
    Tanh = mybir.ActivationFunctionType.Tanh

    import json as _json

    def _dedup_ldweights(b):
        """Remove back-to-back InstLdweights that reload the identical
        stationary operand (the PE array still holds it). Tile emits one
        Ldweights per matmul, so a weight reused by consecutive matmuls is
        loaded twice; each redundant load costs ~53 ns of serial PE time.
        Only sync-free exact duplicates are removed."""
        n_removed = 0
        for fn in b.m.functions:
            for blk in fn.blocks:
                last_key = None
                keep = []
                for inst in blk.instructions:
                    if isinstance(inst, mybir.InstLdweights):
                        d = _json.loads(
                            mybir.instruction_to_pretty_json_string(inst)
                        )
                        si = d.get("sync_info") or {}
                        has_sync = bool(
                            si.get("on_wait") or si.get("on_update")
                        )
                        key = _json.dumps(
                            [
                                d.get("ins"),
                                d.get("perf_mode"),
                                d.get("is_transpose"),
                                d.get("tile_position"),
                                d.get("tile_size"),
                            ],
                            sort_keys=True,
                        )
                        if key == last_key and not has_sync:
                            n_removed += 1
                            continue
                        last_key = key
                    elif isinstance(inst, mybir.InstMatmult):
                        pass  # matmult does not disturb loaded weights
                    elif isinstance(inst, mybir.InstEventSemaphore):
                        pass  # pure semaphore op on the PE queue
                    else:
                        last_key = None
                    keep.append(inst)
                blk.instructions[:] = keep

    class _Bacc(bacc.Bacc):
        def compile(self):
            super().compile()
            _dedup_ldweights(self)

    nc = _Bacc(name="chebykan")
    xt = nc.dram_tensor("xt", (IN_F, B_CORE), f32, kind="ExternalInput")
    w = nc.dram_tensor("w", (DEG, IN_F, OUT_F), bf16, kind="ExternalInput")
    wb = nc.dram_tensor("wb", (P, OUT_F), bf16, kind="ExternalInput")
    out = nc.dram_tensor("out", (B_CORE, OUT_F), f32, kind="ExternalOutput")

    with (
        tile.TileContext(nc) as tc,
        tc.tile_pool(name="wpool", bufs=10) as wpool,
        tc.tile_pool(name="xpool", bufs=8) as xpool,
        tc.tile_pool(name="tanh", bufs=3) as tanpool,
        tc.tile_pool(name="rec", bufs=6) as rpool,
        tc.tile_pool(name="ch", bufs=16) as chpool,
        tc.tile_pool(name="const", bufs=1) as cpool,
        tc.tile_pool(name="outp", bufs=8) as opool,
        tc.tile_pool(name="psum", bufs=1, space="PSUM") as pspool,
    ):
        ones = cpool.tile([P, P], bf16)
        nc.vector.memset(ones[:], 1.0)
        wbias = cpool.tile([P, OUT_F], bf16)
        nc.sync.dma_start(wbias[:], wb[:, :])

        loop_cm = (
            tc.For_i(
                0,
                loop_r,
                1,
                hint_engines=(mybir.EngineType.PE, mybir.EngineType.SP),
            )
            if loop_r is not None
            else contextlib.nullcontext()
        )
        with loop_cm:
            _emit_body(nc, tc, xt, w, out, ones, wbias,
                       wpool, xpool, tanpool, rpool, chpool, opool, pspool,
                       f32, bf16, mult, sub, Tanh, variant)
    nc.finalize()
    return nc


def _emit_body(nc, tc, xt, w, out, ones, wbias,
               wpool, xpool, tanpool, rpool, chpool, opool, pspool,
               f32, bf16, mult, sub, Tanh, variant=""):
    n_oh = 1 if variant == "halfmm" else NOH
    for h in range(N_HALF):
            ps = [
                [
                    pspool.tile(
                        [P, 512], f32, tag=f"ps_{bc}_{oh}", name=f"ps_{bc}_{oh}"
                    )
                    for oh in range(n_oh)
                ]
                for bc in range(NBC)
            ]
            # Bias k-block: out += ones.T @ W_bias (covers the d=0 term).
            # start=True clears the PSUM banks.
            for bc in range(NBC):
                for oh in range(n_oh):
                    nc.tensor.matmul(
                        ps[bc][oh],
                        ones,
                        wbias[:, oh * 512 : (oh + 1) * 512],
                        start=True,
                        stop=False,
                    )
            for i in range(NI):
                xti = xpool.tile([P, HALF], f32, tag="x")
                nc.sync.dma_start(
                    xti[:], xt[i * P : (i + 1) * P, h * HALF : (h + 1) * HALF]
                )
                t = tanpool.tile([P, HALF], f32, tag="t")
                nc.scalar.activation(t[:], xti[:], Tanh)

                tm2 = None  # T_{d-2} (fp32); None encodes T_0 == 1
                tm1 = t  # T_{d-1} (fp32)
                ch1 = None
                for d in range(1, DEG + 1):
                    last = d == DEG
                    if variant == "norec" and d > 1:
                        chd = ch1
                    else:
                        chd = chpool.tile([P, HALF], bf16, tag="ch")
                    if d == 1:
                        nc.scalar.copy(chd[:], t[:])
                        ch1 = chd
                        cur = t
                    elif variant == "norec":
                        cur = None
                    else:
                        # pr = (T_{d-1} * 2) * t  (one fused DVE op)
                        pr = rpool.tile([P, HALF], f32, tag="rec")
                        nc.vector.scalar_tensor_tensor(
                            pr[:], tm1[:], 2.0, t[:], mult, mult
                        )
                        if d == 2:
                            # T_2 = pr - 1
                            cur = rpool.tile([P, HALF], f32, tag="rec")
                            nc.vector.tensor_scalar_sub(cur[:], pr[:], 1.0)
                            nc.scalar.copy(chd[:], cur[:])
                        elif not last:
                            cur = rpool.tile([P, HALF], f32, tag="rec")
                            nc.vector.tensor_tensor(cur[:], pr[:], tm2[:], sub)
                            nc.scalar.copy(chd[:], cur[:])
                        else:
                            # final degree: write the bf16 tile directly
                            cur = None
                            nc.vector.tensor_tensor(chd[:], pr[:], tm2[:], sub)
                    tm2, tm1 = tm1, cur

                    if variant == "nodma":
                        if i == 0 and d == 1:
                            wt0 = wpool.tile([P, 1, OUT_F], bf16, tag="w")
                            nc.sync.dma_start(wt0[:, 0], w[0, 0:P, :])
                        wt = wt0[:, 0]
                    else:
                        wt = wpool.tile([P, OUT_F], bf16, tag="w")
                        nc.sync.dma_start(wt[:], w[d - 1, i * P : (i + 1) * P, :])
                    stop = i == NI - 1 and d == DEG
                    for bc in range(NBC):
                        lhsT = chd[:, bc * P : (bc + 1) * P]
                        for oh in range(n_oh):
                            nc.tensor.matmul(
                                ps[bc][oh],
                                lhsT,
                                wt[:, oh * 512 : (oh + 1) * 512],
                                start=False,
                                stop=stop,
                            )
            # Drain this half's PSUM to SBUF and then HBM. Copies alternate
            # between DVE and ACT to halve the bank-free latency.
            for bc in range(NBC):
                for oh in range(n_oh):
                    ot = opool.tile([P, 512], f32, tag="ot")
                    if (bc * NOH + oh) % 2 == 0:
                        nc.vector.tensor_copy(ot[:], ps[bc][oh])
                    else:
                        nc.scalar.copy(ot[:], ps[bc][oh])
                    r0 = h * HALF + bc * P
                    nc.sync.dma_start(
                        out[r0 : r0 + P, oh * 512 : (oh + 1) * 512], ot[:]
                    )


def _get_nc(loop_r=None, variant=""):
    key = (loop_r, variant)
    if key not in _CACHED_NC:
        _CACHED_NC[key] = _build_bass(loop_r, variant)
    return _CACHED_NC[key]


def _prep_inputs(x, coefficients):
    bf16 = ml_dtypes.bfloat16
    x = np.asarray(x, dtype=np.float32)
    coef = np.asarray(coefficients, dtype=np.float32)
    # (d, i, o) bf16 for d = 1..DEG
    w_all = np.ascontiguousarray(coef.transpose(2, 0, 1)[1 : DEG + 1]).astype(bf16)
    # d=0 term folded over i into a single 128-row contraction block
    wb_arr = np.ascontiguousarray(
        coef[:, :, 0].reshape(NI, P, OUT_F).sum(axis=0)
    ).astype(bf16)
    in_maps = []
    for c in range(N_CORES):
        xc = x[c * B_CORE : (c + 1) * B_CORE, :]
        in_maps.append(
            {
                "xt": np.ascontiguousarray(xc.T),
                "w": w_all,
                "wb": wb_arr,
            }
        )
    return in_maps


def run(x, coefficients, trace=False, tmpdir=None):
    """Run on hardware; returns (out, BassKernelResults)."""
    from concourse.bass_utils import run_bass_kernel_spmd

    nc = _get_nc()
    in_maps = _prep_inputs(x, coefficients)
    res = run_bass_kernel_spmd(
        nc,
        in_maps,
        core_ids=list(range(N_CORES)),
        trace=trace,
        tmpdir=tmpdir,
    )
    out = np.concatenate([r["out"] for r in res.results], axis=0)
    return np.ascontiguousarray(out, dtype=np.float32), res


def kernel(x, coefficients):
    out, _ = run(x, coefficients, trace=False)
    return out
